# revision 57
# baseline (speedup 1.0000x reference)
"""Equivariant layer block (order-2, 15-basis) on 8 Trainium2 NeuronCores.

Decomposition (indices: c in-channel, o out-channel, n/m spatial, N=2048):
  Y[o,n,m] = sum_c X[c,n,m] W8[c,o] + X[c,m,n] W6[c,o]
           + A[o,n] + B[o,m] + D[o,n] delta[n,m]            (+ sum(bias))
with (raw sums; /N factors folded into host-side weights; i = ref basis index)
  A[o,n] = dv.W5 + csum.W7/N + rsum.W12/N + dsum.W11/N + tsum.W14/N^2
  B[o,m] = dv.W9 + csum.W10/N + rsum.W13/N
  D[o,n] = dv.W0 + csum.W1/N + rsum.W3/N + dsum.W2/N + tsum.W4/N^2

v6 design, scheduled around the AllReduce critical path (trigger as early
as possible, then split the output drain between the pre- and post-
collective windows). Core k owns output rows I_k=[256k,256k+256). Both
fp8 spatial panels stay fully resident (128 KB/partition).

Load phase runs ONLY what the collective needs: per chunk, quarter-chunk
DMAs (better queue spread), strided diag extraction, and identity pair-sum
DoubleRow matmuls accumulating partial column sums (row panel) and partial
row sums (column panel) into one PSUM bank. The f16 [128, 770] payload
(pre-folded B table | csum | rsum partials | masked diag col) launches
right after the last chunk lands.

During the collective window the first half of the rows run their main-term
matmuls and are staged to an f16 tile Y0. Post-collective, a short chain
builds A/B/D; one strided Pool add applies D to Y0. The staged half drains
with two DVE 2x-mode broadcast adds in place (+A, +B) and stores f16 (no
PE). The second half runs main matmuls + one-hot B DoubleRow matmuls on
the otherwise idle PE, evicted with +A fused either by Act (per-row
bias=A) or DVE (A broadcast), written fp8, diag-fixed by Pool, stored fp8.
sum(bias) is added on the host (the fp8 half would not survive it).
"""

import os
import numpy as np

import concourse.bacc as bacc
import concourse.tile as tile
import concourse.mybir as mybir
from concourse import bass_utils

N = 2048
C = 16
NCORES = 8
RPC = N // NCORES  # 256 rows per core
G = 8  # m-groups
MW = N // G  # 256
P = 128
CHUNK = 32  # rows per chunk
NCHUNK = RPC // CHUNK  # 8
HB = CHUNK // 2  # row-pairs per chunk (16)
QD = 4  # quarter-chunk DMAs per chunk
QR = 4  # rows per q
NQ = RPC // QR  # 64
SROWS = 120  # staged rows -> f16 path (rest take the fp8 PE path)
BR = 8  # rows per drain block
AXU = 2  # inner expansion of the A table (packed last dim)
CCW = 2 * 256 + 256 + 2  # AllReduce payload cols: B_pre|csum|rsum|dcol|pad
f16 = mybir.dt.float16
f32 = mybir.dt.float32
f8 = mybir.dt.float8e4

LAST_RUN_INFO = {}
_CACHED = {}


def _install_trace_hook():
    """Best-effort NTFF hook injection (used only when BASS_TRACE is set)."""
    try:
        import sys, types

        if "antenv.axon_hooks" in sys.modules:
            return
        mod = types.ModuleType("antenv.axon_hooks")
        state = {}
        mod.set_axon_ntff_profile_hook = lambda h: state.update(h=h)
        mod.get_axon_ntff_profile_hook = lambda: state.get("h")
        sys.modules["antenv.axon_hooks"] = mod
        import antenv

        antenv.axon_hooks = mod
        from trn_agent_boot.trn_boot import _ntff_profile_via_ctypes

        mod.set_axon_ntff_profile_hook(
            _ntff_profile_via_ctypes("/opt/axon/libaxon_pjrt.so")
        )
    except Exception:
        pass


def _build_program():
    nc = bacc.Bacc("TRN2", target_bir_lowering=False, debug=False, num_devices=NCORES)

    # interleaved panel: rc_d[p, b, t, r2, m], row = 2b+r2, t=0 rows / t=1 cols
    rc_d = nc.dram_tensor("rc8", [P, RPC // 2, 2, 2, MW], f8, kind="ExternalInput").ap()
    wst_d = nc.dram_tensor("wst", [P, 2, P], f8, kind="ExternalInput").ap()
    id8_d = nc.dram_tensor("id8", [P, 2, P], f8, kind="ExternalInput").ap()
    ohb_d = nc.dram_tensor("ohb", [P, 2, 2, MW], f8, kind="ExternalInput").ap()
    idt_d = nc.dram_tensor("idt", [P, P], f16, kind="ExternalInput").ap()
    wbcs_d = nc.dram_tensor("wb_cs", [P, P], f32, kind="ExternalInput").ap()
    wbdv_d = nc.dram_tensor("wb_dv", [P, P], f16, kind="ExternalInput").ap()
    wbrs_d = nc.dram_tensor("wb_rs", [P, P], f32, kind="ExternalInput").ap()
    gk16_d = nc.dram_tensor("gk16", [P, C], f16, kind="ExternalInput").ap()
    gall_d = nc.dram_tensor("g_all", [P, C], f16, kind="ExternalInput").ap()
    wad_d = nc.dram_tensor("wad", [2, 96, P], f32, kind="ExternalInput").ap()
    wcc_d = nc.dram_tensor("wcc", [2, 48, P], f32, kind="ExternalInput").ap()
    smask_d = nc.dram_tensor("smask", [P, 1], f32, kind="ExternalInput").ap()

    y16_d = nc.dram_tensor("y16", [P, SROWS, MW], f16, kind="ExternalOutput").ap()
    y8_d = nc.dram_tensor("y8", [P, RPC - SROWS, MW], f8, kind="ExternalOutput").ap()

    add = mybir.AluOpType.add
    COPY = mybir.ActivationFunctionType.Copy
    IDENT = mybir.ActivationFunctionType.Identity

    with tile.TileContext(nc) as tc:
        with (
            tc.tile_pool(name="small", bufs=1) as small,
            tc.tile_pool(name="rcp", bufs=1) as rcp,
            tc.tile_pool(name="y0p", bufs=1) as y0p,
            tc.tile_pool(name="stagep", bufs=2) as stagep,
            tc.tile_pool(name="pscr", bufs=1, space="PSUM") as pscr,
            tc.tile_pool(name="psstat", bufs=1, space="PSUM") as psstat,
            tc.tile_pool(name="psmain", bufs=3, space="PSUM") as psmain,
            tc.tile_pool(name="dram", bufs=1, space="DRAM") as dram,
        ):
            # ---- resident fp8 panels; quarter-chunk DMAs issued from four
            # sequencers in parallel so the queues fill immediately ----
            rc = rcp.tile([P, RPC // 2, 2, 2, MW], f8)
            dma_engs = [nc.sync, nc.scalar, nc.sync, nc.scalar]
            for i in range(NCHUNK):
                b0 = i * HB
                for qd in range(QD):
                    bq = b0 + qd * (HB // QD)
                    dma_engs[qd].dma_start(
                        rc[:, bq : bq + HB // QD, :, :, :],
                        rc_d[:, bq : bq + HB // QD, :, :, :],
                    )
                if i == 0:
                    # small weights after the first chunk's transfers
                    break
            wst = small.tile([P, 2, P], f8)
            id8 = small.tile([P, 2, P], f8)
            ohb = small.tile([P, 2, 2, MW], f8)
            idt = small.tile([P, P], f16)
            wb_cs = small.tile([P, P], f32)
            wb_dv = small.tile([P, P], f16)
            wb_rs = small.tile([P, P], f32)
            gk16 = small.tile([P, C], f16)
            g_all = small.tile([P, C], f16)
            smask = small.tile([P, 1], f32)
            for t, d in [
                (wst, wst_d),
                (id8, id8_d),
                (ohb, ohb_d),
                (idt, idt_d),
                (wb_cs, wbcs_d),
                (wb_dv, wbdv_d),
                (wb_rs, wbrs_d),
                (gk16, gk16_d),
                (g_all, gall_d),
                (smask, smask_d),
            ]:
                nc.gpsimd.dma_start(t[:], d[:])
            wa3 = small.tile([96, P], f32)
            wd3 = small.tile([96, P], f32)
            wca2 = small.tile([48, P], f32)
            wcd2 = small.tile([48, P], f32)
            nc.gpsimd.dma_start(wa3[:], wad_d[0])
            nc.gpsimd.dma_start(wd3[:], wad_d[1])
            nc.gpsimd.dma_start(wca2[:], wcc_d[0])
            nc.gpsimd.dma_start(wcd2[:], wcc_d[1])
            for i in range(1, NCHUNK):
                b0 = i * HB
                for qd in range(QD):
                    bq = b0 + qd * (HB // QD)
                    dma_engs[qd].dma_start(
                        rc[:, bq : bq + HB // QD, :, :, :],
                        rc_d[:, bq : bq + HB // QD, :, :, :],
                    )

            Y0 = y0p.tile([P, SROWS, MW], f16)  # staged main term (pre-tables)
            rdiag = small.tile([P, RPC], f16)  # diag per group (g=k rows valid)
            # csum/rsum pair-sum accumulators share one PSUM bank: [cs | rs]
            csrs = pscr.tile([P, 2 * MW], f32)
            ccbuf = small.tile([P, CCW], f16)
            gbuf = small.tile([P, CCW], f16)
            cc_in = dram.tile([P, CCW], f16)
            cc_out = dram.tile([P, CCW], f16)

            # ---- load phase: stats only (pair-sums + diag extract) ----
            rcflat = rc.rearrange("p b t r m -> p (b t r m)")
            csrs2 = csrs.rearrange("p (t m) -> p t m", t=2)
            for i in range(NCHUNK):
                r0 = i * CHUNK
                b0 = i * HB
                # diag: row r=2b'+r2 at flat offset 1026*b' + 257*r2 + r
                base = b0 * 4 * MW
                for r2i in range(2):
                    o0 = base + 257 * r2i + r0
                    nc.scalar.activation(
                        rdiag[:, r0 + r2i : r0 + CHUNK : 2],
                        rcflat[:, o0 : o0 + (HB - 1) * 1026 + 1 : 1026],
                        COPY,
                    )
                for b in range(HB):
                    gb = b0 + b
                    nc.tensor.matmul(
                        csrs2[:],
                        id8[:],
                        rc[:, gb, :, :, :].rearrange("p t r m -> p r t m"),
                        start=(gb == 0),
                        stop=(gb == RPC // 2 - 1),
                        perf_mode=mybir.MatmulPerfMode.DoubleRow,
                        skip_group_check=True,
                    )

            # ---- pre-collective fold: B_pre + payload assembly ----
            csr32 = small.tile([P, 2 * MW], f32)
            nc.scalar.activation(csr32[:], csrs[:], COPY)
            bps = psstat.tile([P, MW], f32, tag="apck")
            nc.tensor.matmul(bps[:], wb_cs[:], csr32[:, 0:MW], start=True, stop=False)
            nc.tensor.matmul(bps[:], wb_rs[:], csr32[:, MW:], start=False, stop=False)
            nc.tensor.matmul(bps[:], wb_dv[:], rdiag[:], start=False, stop=True)
            with nc.allow_low_precision(reason="f16 collective payload"):
                nc.scalar.activation(ccbuf[:, 0:MW], bps[:], COPY)
                nc.vector.tensor_copy(ccbuf[:, MW : 3 * MW], csr32[:])
                dcol = small.tile([P, 1], f32)
                nc.vector.tensor_reduce(
                    dcol[:], rdiag[:], axis=mybir.AxisListType.X, op=add
                )
                nc.vector.tensor_scalar_mul(
                    ccbuf[:, 3 * MW : 3 * MW + 1], dcol[:], smask[:]
                )
                nc.vector.memset(ccbuf[:, 3 * MW + 1 : CCW], 0.0)
            nc.gpsimd.dma_start(cc_in[:], ccbuf[:])
            # local dv stats fold overlaps the collective
            stats3 = small.tile([96, MW], f32)
            dvp = psstat.tile([P, MW], f32, tag="apck")
            nc.tensor.matmul(dvp[0:C, :], gk16[:], rdiag[:], start=True, stop=True)
            nc.scalar.activation(stats3[0:C, :], dvp[0:C, :], COPY)
            nc.gpsimd.collective_compute(
                "AllReduce",
                add,
                replica_groups=[list(range(NCORES))],
                ins=[cc_in.opt()],
                outs=[cc_out.opt()],
            )
            nc.gpsimd.dma_start(gbuf[:], cc_out[:])

            # ---- staged half: mains + f16 eviction fill the AR window ----
            for q in range(SROWS // QR):
                qr0 = q * QR
                pt = psmain.tile([P, QR, MW], f32, tag="pt")
                for j in range(2):
                    nc.tensor.matmul(
                        pt[:, 2 * j : 2 * j + 2, :],
                        wst[:],
                        rc[:, 2 * q + j, :, :, :],
                        start=True,
                        stop=True,
                        perf_mode=mybir.MatmulPerfMode.DoubleRow,
                    )
                with nc.allow_low_precision(reason="f16 staging"):
                    if q % 2 == 0:
                        nc.scalar.activation(Y0[:, qr0 : qr0 + QR, :], pt[:], COPY)
                    else:
                        nc.vector.tensor_copy(Y0[:, qr0 : qr0 + QR, :], pt[:])

            # ---- post-collective: A/B/D tables ----
            # stats3 rows: 0 dv | 32 csum | 64 rsum; stats2: 0 dsum-b | 32 tsum-b
            g_cs = gbuf[:, MW : 2 * MW]
            g_rs = gbuf[:, 2 * MW : 3 * MW]
            B16 = gbuf[:, 0:MW]  # bias-free B table, used directly
            csp = psstat.tile([P, MW], f32, tag="apck")
            nc.tensor.matmul(csp[0:C, :], gk16[:], g_cs, start=True, stop=True)
            nc.scalar.activation(stats3[32:48, :], csp[0:C, :], COPY)
            rsp = psstat.tile([P, MW], f32, tag="apck")
            nc.tensor.matmul(rsp[0:C, :], gk16[:], g_rs, start=True, stop=True)
            nc.scalar.activation(stats3[64:80, :], rsp[0:C, :], COPY)
            stats2 = small.tile([48, MW], f32)
            dsp = psstat.tile([P, MW], f32, tag="apck")
            nc.tensor.matmul(
                dsp[0:C, 0:1], g_all[:], gbuf[:, 3 * MW : 3 * MW + 1],
                start=True, stop=True,
            )
            nc.vector.tensor_copy(
                stats2[0:C, :], dsp[0:C, 0:1].broadcast_to([C, MW])
            )
            cst2 = small.tile([P, 1], f16)
            with nc.allow_low_precision(reason="f16 total-sum scalar"):
                nc.vector.tensor_reduce(
                    cst2[:], g_cs, axis=mybir.AxisListType.X, op=add
                )
            tsp = psstat.tile([P, MW], f32, tag="apck")
            nc.tensor.matmul(tsp[0:C, 0:1], g_all[:], cst2[:], start=True, stop=True)
            nc.vector.tensor_copy(
                stats2[32:48, :], tsp[0:C, 0:1].broadcast_to([C, MW])
            )

            A16 = small.tile([P, RPC], f16)
            aps = psstat.tile([P, MW], f32, tag="apck")
            nc.tensor.matmul(aps[:], wa3[:], stats3[:], start=True, stop=False)
            nc.tensor.matmul(aps[:], wca2[:], stats2[:], start=False, stop=True)
            with nc.allow_low_precision(reason="f16 A table"):
                nc.scalar.activation(A16[:], aps[:], COPY)
            A8e = small.tile([P, RPC, AXU], f16)
            with nc.allow_low_precision(reason="f16 A table"):
                nc.vector.tensor_copy(
                    A8e[:],
                    A16.rearrange("p (n x) -> p n x", x=1).broadcast_to(
                        [P, RPC, AXU]
                    ),
                )
            Dm16 = small.tile([P, RPC], f16)
            dps = psstat.tile([P, MW], f32, tag="apck")
            nc.tensor.matmul(dps[:], wd3[:], stats3[:], start=True, stop=False)
            nc.tensor.matmul(dps[:], wcd2[:], stats2[:], start=False, stop=True)
            with nc.allow_low_precision(reason="f16 D table"):
                nc.scalar.activation(Dm16[:], dps[:], COPY)
            # bt8: fp8 transpose of the B table, x32 scaled so the small B
            # values stay in fp8e4m3 normal range (ohb carries 1/32)
            bt8 = small.tile([P, 2, P], f8)
            for mb in range(2):
                btp = psstat.tile([P, P], f16, tag="apck")
                nc.tensor.matmul(
                    btp[:],
                    gbuf[:, mb * P : (mb + 1) * P],
                    idt[:],
                    is_transpose=True,
                    start=True,
                    stop=True,
                    skip_group_check=True,
                )
                with nc.allow_low_precision(reason="fp8 B table"):
                    nc.scalar.activation(bt8[:, mb, :], btp[:], COPY, scale=32.0)

            # ---- staged-half diag fix: one strided add over Y0 ----
            y0flat = Y0.rearrange("p n m -> p (n m)")
            with nc.allow_low_precision(reason="f16 output"):
                nc.gpsimd.tensor_tensor(
                    y0flat[:, 0 : (SROWS - 1) * (MW + 1) + 1 : MW + 1],
                    y0flat[:, 0 : (SROWS - 1) * (MW + 1) + 1 : MW + 1],
                    Dm16[:, 0:SROWS],
                    op=add,
                )

            # ---- drain: staged half on DVE (2x adds in place, f16 store);
            # unstaged half on PE (+one-hot B) with Act/DVE +A eviction ----
            ustart = SROWS // QR  # first unstaged q
            dve_helper = {q for q in range(ustart, NQ) if q % 8 in (3, 7)}
            for blk in range(NQ - ustart):
                # one staged block (8 rows) + one unstaged block (4 rows),
                # interleaved so all engines stay fed
                if blk < SROWS // BR:
                    r0 = blk * BR
                    y0b = Y0[:, r0 : r0 + BR, :]
                    y0b4 = y0b.rearrange("p n (u x) -> p n u x", x=AXU)
                    with nc.allow_low_precision(reason="f16 output"):
                        nc.vector.tensor_tensor(
                            y0b4[:],
                            y0b4[:],
                            A8e[:, r0 : r0 + BR, :]
                            .rearrange("p n (u x) -> p n u x", u=1)
                            .broadcast_to([P, BR, MW // AXU, AXU]),
                            op=add,
                        )
                        nc.vector.tensor_tensor(
                            y0b[:],
                            y0b[:],
                            B16.rearrange("p (n m) -> p n m", n=1).broadcast_to(
                                [P, BR, MW]
                            ),
                            op=add,
                        )
                    nc.sync.dma_start(y16_d[:, r0 : r0 + BR, :], y0b[:])
                q = ustart + blk
                if q < NQ:
                    qr0 = q * QR
                    ur0 = qr0 - SROWS  # row offset within the fp8 output
                    pt = psmain.tile([P, QR, MW], f32, tag="pt")
                    for j in range(2):
                        nc.tensor.matmul(
                            pt[:, 2 * j : 2 * j + 2, :],
                            wst[:],
                            rc[:, 2 * q + j, :, :, :],
                            start=True,
                            stop=False,
                            perf_mode=mybir.MatmulPerfMode.DoubleRow,
                        )
                    for j in range(2):
                        nc.tensor.matmul(
                            pt[:, 2 * j : 2 * j + 2, :],
                            bt8[:],
                            ohb[:],
                            start=False,
                            stop=(j == 1),
                            perf_mode=mybir.MatmulPerfMode.DoubleRow,
                            skip_group_check=True,
                        )
                    st = stagep.tile([P, QR, MW], f8, tag="st")
                    with nc.allow_low_precision(reason="fp8 output"):
                        if q in dve_helper:
                            st4 = st.rearrange("p n (u x) -> p n u x", x=AXU)
                            nc.vector.tensor_tensor(
                                st4[:],
                                pt.rearrange("p n (u x) -> p n u x", x=AXU),
                                A8e[:, qr0 : qr0 + QR, :]
                                .rearrange("p n (u x) -> p n u x", u=1)
                                .broadcast_to([P, QR, MW // AXU, AXU]),
                                op=add,
                            )
                        else:
                            for r in range(QR):
                                nc.scalar.activation(
                                    st[:, r, :],
                                    pt[:, r, :],
                                    IDENT,
                                    bias=A16[:, qr0 + r : qr0 + r + 1],
                                )
                        # fp8-half diag fix on the stage tile
                        stflat = st.rearrange("p n m -> p (n m)")
                        nc.gpsimd.tensor_tensor(
                            stflat[:, qr0 : qr0 + (QR - 1) * (MW + 1) + 1 : MW + 1],
                            stflat[:, qr0 : qr0 + (QR - 1) * (MW + 1) + 1 : MW + 1],
                            Dm16[:, qr0 : qr0 + QR],
                            op=add,
                        )
                    nc.sync.dma_start(y8_d[:, ur0 : ur0 + QR, :], st[:])

    nc.compile()
    return nc


def _host_prep(X, weights, bias):
    """Pack panels + fold weights into per-core input maps."""
    import ml_dtypes

    W = weights.astype(np.float32)
    iN = np.float32(1.0 / N)
    iN2 = np.float32(1.0 / (N * N))

    Xr = np.ascontiguousarray(X[0])  # [C, N, N] fp32
    Rp = (
        Xr.reshape(C, NCORES, RPC, G, MW)
        .transpose(1, 3, 0, 2, 4)
        .reshape(NCORES, P, RPC, MW)
        .astype(ml_dtypes.float8_e4m3)
    )
    XT = np.ascontiguousarray(Xr.transpose(0, 2, 1))
    Cp = (
        XT.reshape(C, NCORES, RPC, G, MW)
        .transpose(1, 3, 0, 2, 4)
        .reshape(NCORES, P, RPC, MW)
        .astype(ml_dtypes.float8_e4m3)
    )
    # interleave: rc8[k, p, b, t, r2, m], row = 2b + r2; t=0 rows, t=1 cols
    rc8 = np.ascontiguousarray(
        np.stack(
            [
                Rp.reshape(NCORES, P, RPC // 2, 2, MW),
                Cp.reshape(NCORES, P, RPC // 2, 2, MW),
            ],
            axis=3,
        )
    )

    def blockdiag(w, dtype):
        out = np.zeros((P, P), dtype=dtype)
        for g in range(G):
            out[g * C : (g + 1) * C, g * C : (g + 1) * C] = w
        return out

    wst = np.stack(
        [
            blockdiag(W[8], np.float32),
            blockdiag(W[6], np.float32),
        ],
        axis=1,
    ).astype(ml_dtypes.float8_e4m3)  # [128, 2, 128]

    id8 = np.stack([np.eye(P, dtype=np.float32)] * 2, axis=1).astype(
        ml_dtypes.float8_e4m3
    )  # [128, 2, 128] stacked identities for the pair-sum matmuls
    idt = np.eye(P, dtype=np.float16)
    # one-hot B pattern (x 1/32; bt8 is x32): ohb[k, t, r, m] = (m == 128t+k)/32
    ohb = np.zeros((P, 2, 2, MW), np.float32)
    for t in range(2):
        for k in range(P):
            ohb[k, t, :, t * P + k] = 1.0 / 32.0
    ohb = ohb.astype(ml_dtypes.float8_e4m3)
    wb_cs = blockdiag(W[10] * iN, np.float32)
    wb_rs = blockdiag(W[13] * iN, np.float32)
    g_all = np.tile(np.eye(C, dtype=np.float32), (G, 1))  # [128, 16]

    def rep(w):  # [16,16] -> [16, 128]
        return np.tile(w.astype(np.float32), (1, G))

    in_maps = []
    for k in range(NCORES):
        rowmask = np.repeat((np.arange(G) == k).astype(np.float32), C)  # [128]
        gk = g_all * rowmask[:, None]
        wb_dv = blockdiag(W[9], np.float32) * rowmask[:, None]
        # stats3 rows: 0 dv | 32 csum | 64 rsum; stats2: 0 dsum-b | 32 tsum-b
        wad = np.zeros((2, 96, P), np.float32)
        wad[0, 0:C] = rep(W[5])
        wad[0, 32:48] = rep(W[7] * iN)
        wad[0, 64:80] = rep(W[12] * iN)
        wad[1, 0:C] = rep(W[0]) * rowmask[None, :]
        wad[1, 32:48] = rep(W[1] * iN) * rowmask[None, :]
        wad[1, 64:80] = rep(W[3] * iN) * rowmask[None, :]
        wcc = np.zeros((2, 48, P), np.float32)
        wcc[0, 0:C] = rep(W[11] * iN)
        wcc[0, 32:48] = rep(W[14] * iN2)
        wcc[1, 0:C] = rep(W[2] * iN) * rowmask[None, :]
        wcc[1, 32:48] = rep(W[4] * iN2) * rowmask[None, :]
        in_maps.append(
            {
                "rc8": rc8[k],
                "wst": wst,
                "id8": id8,
                "idt": idt,
                "ohb": ohb,
                "wb_cs": wb_cs,
                "wb_dv": wb_dv.astype(np.float16),
                "wb_rs": wb_rs,
                "gk16": gk.astype(np.float16),
                "g_all": g_all.astype(np.float16),
                "wad": wad,
                "wcc": wcc,
                "smask": rowmask[:, None].copy(),
            }
        )
    return in_maps


def kernel(X, weights, bias):
    if "nc" not in _CACHED:
        _CACHED["nc"] = _build_program()
    nc = _CACHED["nc"]

    trace = bool(os.environ.get("BASS_TRACE"))
    if trace:
        _install_trace_hook()

    in_maps = _host_prep(np.asarray(X), np.asarray(weights), np.asarray(bias))
    res = bass_utils.run_bass_kernel_spmd(
        nc, in_maps, core_ids=list(range(NCORES)), trace=trace
    )
    LAST_RUN_INFO.clear()
    LAST_RUN_INFO.update(
        exec_time_ns=res.exec_time_ns,
        mean_exec_time_ns=res.mean_exec_time_ns,
        trace=res.instructions_and_trace[1] if res.instructions_and_trace else None,
    )

    bias_sum = np.float32(np.asarray(bias).astype(np.float64).sum())
    Yp = np.concatenate(
        [
            np.stack(
                [np.asarray(res.results[k]["y16"]) for k in range(NCORES)]
            ).astype(np.float32),
            np.stack(
                [np.asarray(res.results[k]["y8"]) for k in range(NCORES)]
            ).astype(np.float32),
        ],
        axis=2,
    )
    Y = (
        (Yp + bias_sum)
        .reshape(NCORES, G, C, RPC, MW)
        .transpose(2, 0, 3, 1, 4)
        .reshape(1, C, N, N)
    )
    return Y


# revision 58
# speedup vs baseline: 1.0626x; 1.0626x over previous
"""Equivariant layer block (order-2, 15-basis) on 8 Trainium2 NeuronCores.

Decomposition (indices: c in-channel, o out-channel, n/m spatial, N=2048):
  Y[o,n,m] = sum_c X[c,n,m] W8[c,o] + X[c,m,n] W6[c,o]
           + A[o,n] + B[o,m] + D[o,n] delta[n,m]
with (raw sums; /N factors folded into host-side weights; i = ref basis index)
  A[o,n] = dv.W5 + csum.W7/N + rsum.W12/N + dsum.W11/N + tsum.W14/N^2
  B[o,m] = dv.W9 + csum.W10/N + rsum.W13/N + sum(bias)
  D[o,n] = dv.W0 + csum.W1/N + rsum.W3/N + dsum.W2/N + tsum.W4/N^2

v4 design. Core k owns output rows I_k=[256k,256k+256). Both spatial panels
stream through a 4-chunk fp8 SBUF ring rc[(g,c), b, t, r2, m] (t=0 rows,
t=1 cols); per chunk the PE does the fused DoubleRow main matmul (identity +
transpose terms via the two k-tiles) plus identity pair-sum matmuls that
accumulate partial column sums (from the row panel) AND partial row sums
(from the column panel) in one persistent PSUM bank. The main term is
evicted table-free into a full f16 staging tile Y0 (Act/DVE alternating
plain copies), so the whole load phase is DMA-bound and the PE never
switches stationary weights mid-phase.

One f16 AllReduce of [128, 770] (pre-folded B table | csum partials | rsum
partials | masked diag column) fires as soon as the last chunk's pair-sums
land. Post-collective, a short matmul chain builds the A/B/D tables, then
pass 2 walks Y0 in 8-row blocks: two DVE 4x-mode broadcast adds (+A along
m, +B along r), a tiny Pool diag fix-up, and pipelined f16 stores on the
sync/gpsimd queues. sum(bias) is folded into B on device.
"""

import os
import numpy as np

import concourse.bacc as bacc
import concourse.tile as tile
import concourse.mybir as mybir
from concourse import bass_utils

N = 2048
C = 16
NCORES = 8
RPC = N // NCORES  # 256 rows per core
G = 8  # m-groups
MW = N // G  # 256
P = 128
CHUNK = 32  # rows per DMA chunk
NCHUNK = RPC // CHUNK  # 8
HB = CHUNK // 2  # row-pairs per chunk (16)
QR = 4  # rows per main-loop quarter-chunk
NQ = RPC // QR  # 64
BR = 8  # rows per pass-2 block
NB = RPC // BR  # 32
AXU = 8  # inner expansion of the A table (packed last dim for DVE 2x)
CCW = 2 * 256 + 256 + 2  # AllReduce payload cols: B_pre|csum|rsum|dcol|pad
f16 = mybir.dt.float16
f32 = mybir.dt.float32
f8 = mybir.dt.float8e4

LAST_RUN_INFO = {}
_CACHED = {}


def _install_trace_hook():
    """Best-effort NTFF hook injection (used only when BASS_TRACE is set)."""
    try:
        import sys, types

        if "antenv.axon_hooks" in sys.modules:
            return
        mod = types.ModuleType("antenv.axon_hooks")
        state = {}
        mod.set_axon_ntff_profile_hook = lambda h: state.update(h=h)
        mod.get_axon_ntff_profile_hook = lambda: state.get("h")
        sys.modules["antenv.axon_hooks"] = mod
        import antenv

        antenv.axon_hooks = mod
        from trn_agent_boot.trn_boot import _ntff_profile_via_ctypes

        mod.set_axon_ntff_profile_hook(
            _ntff_profile_via_ctypes("/opt/axon/libaxon_pjrt.so")
        )
    except Exception:
        pass


def _build_program():
    nc = bacc.Bacc("TRN2", target_bir_lowering=False, debug=False, num_devices=NCORES)

    # interleaved panel: rc_d[p, b, t, r2, m], row = 2b+r2, t=0 rows / t=1 cols
    rc_d = nc.dram_tensor("rc8", [P, RPC // 2, 2, 2, MW], f8, kind="ExternalInput").ap()
    wst_d = nc.dram_tensor("wst", [P, 2, P], f8, kind="ExternalInput").ap()
    id8_d = nc.dram_tensor("id8", [P, 2, P], f8, kind="ExternalInput").ap()
    ohb_d = nc.dram_tensor("ohb", [P, 2, 2, MW], f8, kind="ExternalInput").ap()
    idt_d = nc.dram_tensor("idt", [P, P], f16, kind="ExternalInput").ap()
    wbcs_d = nc.dram_tensor("wb_cs", [P, P], f32, kind="ExternalInput").ap()
    wbdv_d = nc.dram_tensor("wb_dv", [P, P], f16, kind="ExternalInput").ap()
    wbrs_d = nc.dram_tensor("wb_rs", [P, P], f32, kind="ExternalInput").ap()
    gk16_d = nc.dram_tensor("gk16", [P, C], f16, kind="ExternalInput").ap()
    gall_d = nc.dram_tensor("g_all", [P, C], f16, kind="ExternalInput").ap()
    wad_d = nc.dram_tensor("wad", [2, 96, P], f32, kind="ExternalInput").ap()
    wcc_d = nc.dram_tensor("wcc", [2, 48, P], f32, kind="ExternalInput").ap()
    smask_d = nc.dram_tensor("smask", [P, 1], f32, kind="ExternalInput").ap()
    bsum_d = nc.dram_tensor("bsum", [P, 1], f32, kind="ExternalInput").ap()

    y_d = nc.dram_tensor("y", [P, RPC, MW], f16, kind="ExternalOutput").ap()

    add = mybir.AluOpType.add
    COPY = mybir.ActivationFunctionType.Copy
    IDENT = mybir.ActivationFunctionType.Identity

    with tile.TileContext(nc) as tc:
        with (
            tc.tile_pool(name="small", bufs=1) as small,
            tc.tile_pool(name="rcring", bufs=3) as rcring,
            tc.tile_pool(name="y0p", bufs=1) as y0p,
            tc.tile_pool(name="pscr", bufs=1, space="PSUM") as pscr,
            tc.tile_pool(name="psstat", bufs=1, space="PSUM") as psstat,
            tc.tile_pool(name="psmain", bufs=3, space="PSUM") as psmain,
            tc.tile_pool(name="dram", bufs=1, space="DRAM") as dram,
        ):
            # ---- kick the first input chunks before the small weights ----
            rcts = []
            for i in range(NCHUNK):
                rct = rcring.tile([P, HB, 2, 2, MW], f8, tag="rc", name=f"rc{i}")
                rcts.append(rct)
            for i in range(2):
                nc.sync.dma_start(
                    rcts[i][:], rc_d[:, i * HB : (i + 1) * HB, :, :, :]
                )

            # ---- constant / weight loads ----
            wst = small.tile([P, 2, P], f8)
            id8 = small.tile([P, 2, P], f8)
            ohb = small.tile([P, 2, 2, MW], f8)
            idt = small.tile([P, P], f16)
            wb_cs = small.tile([P, P], f32)
            wb_dv = small.tile([P, P], f16)
            wb_rs = small.tile([P, P], f32)
            gk16 = small.tile([P, C], f16)
            g_all = small.tile([P, C], f16)
            smask = small.tile([P, 1], f32)
            bsum = small.tile([P, 1], f32)
            for t, d in [
                (wst, wst_d),
                (id8, id8_d),
                (ohb, ohb_d),
                (idt, idt_d),
                (wb_cs, wbcs_d),
                (wb_dv, wbdv_d),
                (wb_rs, wbrs_d),
                (gk16, gk16_d),
                (g_all, gall_d),
                (smask, smask_d),
                (bsum, bsum_d),
            ]:
                nc.sync.dma_start(t[:], d[:])
            wa3 = small.tile([96, P], f32)
            wd3 = small.tile([96, P], f32)
            wca2 = small.tile([48, P], f32)
            wcd2 = small.tile([48, P], f32)
            nc.sync.dma_start(wa3[:], wad_d[0])
            nc.sync.dma_start(wd3[:], wad_d[1])
            nc.sync.dma_start(wca2[:], wcc_d[0])
            nc.sync.dma_start(wcd2[:], wcc_d[1])

            Y0 = y0p.tile([P, RPC, MW], f16)  # staged main term (pre-tables)
            rdiag = small.tile([P, RPC], f16)  # diag per group (g=k rows valid)
            # csum/rsum pair-sum accumulators share one PSUM bank: [cs | rs]
            csrs = pscr.tile([P, 2 * MW], f32)
            ccbuf = small.tile([P, CCW], f16)
            gbuf = small.tile([P, CCW], f16)
            cc_in = dram.tile([P, CCW], f16)
            cc_out = dram.tile([P, CCW], f16)

            # ---- load phase: stream chunks, stats + main matmuls + evict ----
            for i in range(NCHUNK):
                r0 = i * CHUNK
                b0 = i * HB
                rct = rcts[i]
                if i >= 2:
                    nc.sync.dma_start(rct[:], rc_d[:, b0 : b0 + HB, :, :, :])
                # diag extract: row r=2b'+r2 at flat offset 1026*b' + 257*r2
                rcflat = rct.rearrange("p b t r m -> p (b t r m)")
                for r2i in range(2):
                    nc.scalar.activation(
                        rdiag[:, r0 + r2i : r0 + CHUNK : 2],
                        rcflat[
                            :,
                            257 * r2i + r0 : 257 * r2i + r0 + (HB - 1) * 1026 + 1 : 1026,
                        ],
                        COPY,
                    )
                # pair-sum stats on PE: one DoubleRow matmul per row-pair
                # covers BOTH panels (pair over r2; free dims [t, m]):
                # csrs[(g,c), t*MW+m] += rct[.., b, t, 0, m] + rct[.., b, t, 1, m]
                csrs2 = csrs.rearrange("p (t m) -> p t m", t=2)
                for b in range(HB):
                    gb = b0 + b
                    nc.tensor.matmul(
                        csrs2[:],
                        id8[:],
                        rct[:, b, :, :, :].rearrange("p t r m -> p r t m"),
                        start=(gb == 0),
                        stop=(gb == RPC // 2 - 1),
                        perf_mode=mybir.MatmulPerfMode.DoubleRow,
                        skip_group_check=True,
                    )
                # main term: one DoubleRow matmul per row-pair (pair over t)
                for s in range(CHUNK // QR):
                    qr0 = r0 + s * QR
                    pt = psmain.tile([P, QR, MW], f32, tag="pt")
                    for j in range(2):
                        nc.tensor.matmul(
                            pt[:, 2 * j : 2 * j + 2, :],
                            wst[:],
                            rct[:, 2 * s + j, :, :, :],
                            start=True,
                            stop=True,
                            perf_mode=mybir.MatmulPerfMode.DoubleRow,
                        )
                    with nc.allow_low_precision(reason="f16 staging"):
                        if s % 2 == 0:
                            nc.scalar.activation(
                                Y0[:, qr0 : qr0 + QR, :], pt[:], COPY
                            )
                        else:
                            nc.vector.tensor_copy(Y0[:, qr0 : qr0 + QR, :], pt[:])

            # ---- pre-collective fold: B_pre + payload assembly ----
            csr32 = small.tile([P, 2 * MW], f32)
            nc.scalar.activation(csr32[:], csrs[:], COPY)
            bps = psstat.tile([P, MW], f32, tag="apck")
            nc.tensor.matmul(bps[:], wb_cs[:], csr32[:, 0:MW], start=True, stop=False)
            nc.tensor.matmul(bps[:], wb_rs[:], csr32[:, MW:], start=False, stop=False)
            nc.tensor.matmul(bps[:], wb_dv[:], rdiag[:], start=False, stop=True)
            with nc.allow_low_precision(reason="f16 collective payload"):
                nc.scalar.activation(ccbuf[:, 0:MW], bps[:], COPY)
                nc.vector.tensor_copy(ccbuf[:, MW : 3 * MW], csr32[:])
                dcol = small.tile([P, 1], f32)
                nc.vector.tensor_reduce(
                    dcol[:], rdiag[:], axis=mybir.AxisListType.X, op=add
                )
                nc.vector.tensor_scalar_mul(
                    ccbuf[:, 3 * MW : 3 * MW + 1], dcol[:], smask[:]
                )
                nc.vector.memset(ccbuf[:, 3 * MW + 1 : CCW], 0.0)
            nc.gpsimd.dma_start(cc_in[:], ccbuf[:])
            # local dv stats fold overlaps the collective
            stats3 = small.tile([96, MW], f32)
            dvp = psstat.tile([P, MW], f32, tag="apck")
            nc.tensor.matmul(dvp[0:C, :], gk16[:], rdiag[:], start=True, stop=True)
            nc.scalar.activation(stats3[0:C, :], dvp[0:C, :], COPY)
            nc.gpsimd.collective_compute(
                "AllReduce",
                add,
                replica_groups=[list(range(NCORES))],
                ins=[cc_in.opt()],
                outs=[cc_out.opt()],
            )
            nc.gpsimd.dma_start(gbuf[:], cc_out[:])

            # ---- post-collective: A/B/D tables ----
            # stats3 rows: 0 dv | 16 csum | 32 rsum | 48 dsum-bcast | 64 tsum-b
            g_cs = gbuf[:, MW : 2 * MW]
            g_rs = gbuf[:, 2 * MW : 3 * MW]
            B16 = small.tile([P, MW], f16)
            with nc.allow_low_precision(reason="f16 B table"):
                nc.scalar.activation(B16[:], gbuf[:, 0:MW], IDENT, bias=bsum[:])
            csp = psstat.tile([P, MW], f32, tag="apck")
            nc.tensor.matmul(csp[0:C, :], gk16[:], g_cs, start=True, stop=True)
            nc.scalar.activation(stats3[32:48, :], csp[0:C, :], COPY)
            rsp = psstat.tile([P, MW], f32, tag="apck")
            nc.tensor.matmul(rsp[0:C, :], gk16[:], g_rs, start=True, stop=True)
            nc.scalar.activation(stats3[64:80, :], rsp[0:C, :], COPY)
            # stats2: dsum/tsum broadcast rows (32-aligned partition bases)
            stats2 = small.tile([48, MW], f32)
            dsp = psstat.tile([P, MW], f32, tag="apck")
            nc.tensor.matmul(
                dsp[0:C, 0:1], g_all[:], gbuf[:, 3 * MW : 3 * MW + 1],
                start=True, stop=True,
            )
            nc.vector.tensor_copy(
                stats2[0:C, :], dsp[0:C, 0:1].broadcast_to([C, MW])
            )
            cst2 = small.tile([P, 1], f16)
            with nc.allow_low_precision(reason="f16 total-sum scalar"):
                nc.vector.tensor_reduce(
                    cst2[:], g_cs, axis=mybir.AxisListType.X, op=add
                )
            tsp = psstat.tile([P, MW], f32, tag="apck")
            nc.tensor.matmul(tsp[0:C, 0:1], g_all[:], cst2[:], start=True, stop=True)
            nc.vector.tensor_copy(
                stats2[32:48, :], tsp[0:C, 0:1].broadcast_to([C, MW])
            )

            A16 = small.tile([P, RPC], f16)
            aps = psstat.tile([P, MW], f32, tag="apck")
            nc.tensor.matmul(aps[:], wa3[:], stats3[:], start=True, stop=False)
            nc.tensor.matmul(aps[:], wca2[:], stats2[:], start=False, stop=True)
            with nc.allow_low_precision(reason="f16 A table"):
                nc.scalar.activation(A16[:], aps[:], COPY)
            # A expanded AXU-wide so pass-2 keeps a packed last dim (DVE 4x)
            A8e = small.tile([P, RPC, AXU], f16)
            with nc.allow_low_precision(reason="f16 A table"):
                nc.vector.tensor_copy(
                    A8e[:],
                    A16.rearrange("p (n x) -> p n x", x=1).broadcast_to(
                        [P, RPC, AXU]
                    ),
                )
            Dm16 = small.tile([P, RPC], f16)
            dps = psstat.tile([P, MW], f32, tag="apck")
            nc.tensor.matmul(dps[:], wd3[:], stats3[:], start=True, stop=False)
            nc.tensor.matmul(dps[:], wcd2[:], stats2[:], start=False, stop=True)
            with nc.allow_low_precision(reason="f16 D table"):
                nc.scalar.activation(Dm16[:], dps[:], COPY)
            # A16b = A + sum(bias): per-row bias for the PE-path evictions
            A16b = small.tile([P, RPC], f16)
            with nc.allow_low_precision(reason="f16 A table"):
                nc.scalar.activation(A16b[:], A16[:], IDENT, bias=bsum[:])
            # bt8: fp8 transpose of the bias-free B table, x32 scaled so the
            # small B values stay in fp8e4m3 normal range (ohb carries 1/32)
            bt8 = small.tile([P, 2, P], f8)
            for mb in range(2):
                btp = psstat.tile([P, P], f16, tag="apck")
                nc.tensor.matmul(
                    btp[:],
                    gbuf[:, mb * P : (mb + 1) * P],
                    idt[:],
                    is_transpose=True,
                    start=True,
                    stop=True,
                    skip_group_check=True,
                )
                with nc.allow_low_precision(reason="fp8 B table"):
                    nc.scalar.activation(bt8[:, mb, :], btp[:], COPY, scale=32.0)

            # ---- diag fix-up: one strided add over ALL of Y0 (row r diag
            # sits at flat r*257 within [r, m']; Dm16 is zero off g=k) ----
            y0flat = Y0.rearrange("p n m -> p (n m)")
            with nc.allow_low_precision(reason="f16 output"):
                nc.gpsimd.tensor_tensor(
                    y0flat[:, 0 : (RPC - 1) * (MW + 1) + 1 : MW + 1],
                    y0flat[:, 0 : (RPC - 1) * (MW + 1) + 1 : MW + 1],
                    Dm16[:],
                    op=add,
                )

            # ---- pass 2: +A (bcast m), +B (bcast r) in place on Y0, store --
            # DVE-path blocks: two 2x-mode f16 adds. PE-path blocks: identity
            # copy-matmul + one-hot B matmul into PSUM, Act per-row bias=A
            # eviction. Split keeps the tail under the store-DMA bound.
            pe_path = {qq for qq in range(NB) if qq % 5 in (1, 3)}
            for qq in range(NB):
                r0 = qq * BR
                y0b = Y0[:, r0 : r0 + BR, :]
                if qq in pe_path:
                    # PE path: 2 q-subblocks of 4 rows
                    pts = []
                    for j in range(2):
                        pt2 = psmain.tile([P, QR, MW], f32, tag="pt")
                        for h in range(2):
                            nc.tensor.matmul(
                                pt2[:, 2 * h : 2 * h + 2, :],
                                idt[:],
                                y0b[:, 4 * j + 2 * h : 4 * j + 2 * h + 2, :],
                                start=True,
                                stop=False,
                                skip_group_check=True,
                            )
                        pts.append(pt2)
                    for j in range(2):
                        for h in range(2):
                            nc.tensor.matmul(
                                pts[j][:, 2 * h : 2 * h + 2, :],
                                bt8[:],
                                ohb[:],
                                start=False,
                                stop=True,
                                perf_mode=mybir.MatmulPerfMode.DoubleRow,
                                skip_group_check=True,
                            )
                    with nc.allow_low_precision(reason="f16 output"):
                        for r in range(BR):
                            nc.scalar.activation(
                                y0b[:, r, :],
                                pts[r // 4][:, r % 4, :],
                                IDENT,
                                bias=A16b[:, r0 + r : r0 + r + 1],
                            )
                else:
                    y0b4 = y0b.rearrange("p n (u x) -> p n u x", x=AXU)
                    with nc.allow_low_precision(reason="f16 output"):
                        nc.vector.tensor_tensor(
                            y0b4[:],
                            y0b4[:],
                            A8e[:, r0 : r0 + BR, :]
                            .rearrange("p n (u x) -> p n u x", u=1)
                            .broadcast_to([P, BR, MW // AXU, AXU]),
                            op=add,
                        )
                        nc.vector.tensor_tensor(
                            y0b[:],
                            y0b[:],
                            B16.rearrange("p (n m) -> p n m", n=1).broadcast_to(
                                [P, BR, MW]
                            ),
                            op=add,
                        )
                nc.sync.dma_start(y_d[:, r0 : r0 + BR, :], y0b[:])

    nc.compile()
    return nc


def _host_prep(X, weights, bias):
    """Pack panels + fold weights into per-core input maps."""
    import ml_dtypes

    W = weights.astype(np.float32)
    iN = np.float32(1.0 / N)
    iN2 = np.float32(1.0 / (N * N))
    bias_sum = np.float32(bias.astype(np.float64).sum())

    Xr = np.ascontiguousarray(X[0])  # [C, N, N] fp32
    Rp = (
        Xr.reshape(C, NCORES, RPC, G, MW)
        .transpose(1, 3, 0, 2, 4)
        .reshape(NCORES, P, RPC, MW)
        .astype(ml_dtypes.float8_e4m3)
    )
    XT = np.ascontiguousarray(Xr.transpose(0, 2, 1))
    Cp = (
        XT.reshape(C, NCORES, RPC, G, MW)
        .transpose(1, 3, 0, 2, 4)
        .reshape(NCORES, P, RPC, MW)
        .astype(ml_dtypes.float8_e4m3)
    )
    # interleave: rc8[k, p, b, t, r2, m], row = 2b + r2; t=0 rows, t=1 cols
    rc8 = np.ascontiguousarray(
        np.stack(
            [
                Rp.reshape(NCORES, P, RPC // 2, 2, MW),
                Cp.reshape(NCORES, P, RPC // 2, 2, MW),
            ],
            axis=3,
        )
    )

    def blockdiag(w, dtype):
        out = np.zeros((P, P), dtype=dtype)
        for g in range(G):
            out[g * C : (g + 1) * C, g * C : (g + 1) * C] = w
        return out

    wst = np.stack(
        [
            blockdiag(W[8], np.float32),
            blockdiag(W[6], np.float32),
        ],
        axis=1,
    ).astype(ml_dtypes.float8_e4m3)  # [128, 2, 128]

    id8 = np.stack([np.eye(P, dtype=np.float32)] * 2, axis=1).astype(
        ml_dtypes.float8_e4m3
    )  # [128, 2, 128] stacked identities for the pair-sum matmuls
    idt = np.eye(P, dtype=np.float16)
    # one-hot B pattern (x 1/32; bt8 is x32): ohb[k, t, r, m] = (m == 128t+k)/32
    ohb = np.zeros((P, 2, 2, N // G), np.float32)
    for t in range(2):
        for k in range(P):
            ohb[k, t, :, t * P + k] = 1.0 / 32.0
    ohb = ohb.astype(ml_dtypes.float8_e4m3)
    wb_cs = blockdiag(W[10] * iN, np.float32)
    wb_rs = blockdiag(W[13] * iN, np.float32)
    g_all = np.tile(np.eye(C, dtype=np.float32), (G, 1))  # [128, 16]

    def rep(w):  # [16,16] -> [16, 128]
        return np.tile(w.astype(np.float32), (1, G))

    in_maps = []
    for k in range(NCORES):
        rowmask = np.repeat((np.arange(G) == k).astype(np.float32), C)  # [128]
        gk = g_all * rowmask[:, None]
        wb_dv = blockdiag(W[9], np.float32) * rowmask[:, None]
        # stats3 rows: 0 dv | 32 csum | 64 rsum; stats2: 0 dsum-b | 32 tsum-b
        wad = np.zeros((2, 96, P), np.float32)
        wad[0, 0:C] = rep(W[5])
        wad[0, 32:48] = rep(W[7] * iN)
        wad[0, 64:80] = rep(W[12] * iN)
        wad[1, 0:C] = rep(W[0]) * rowmask[None, :]
        wad[1, 32:48] = rep(W[1] * iN) * rowmask[None, :]
        wad[1, 64:80] = rep(W[3] * iN) * rowmask[None, :]
        wcc = np.zeros((2, 48, P), np.float32)
        wcc[0, 0:C] = rep(W[11] * iN)
        wcc[0, 32:48] = rep(W[14] * iN2)
        wcc[1, 0:C] = rep(W[2] * iN) * rowmask[None, :]
        wcc[1, 32:48] = rep(W[4] * iN2) * rowmask[None, :]
        in_maps.append(
            {
                "rc8": rc8[k],
                "wst": wst,
                "id8": id8,
                "idt": idt,
                "ohb": ohb,
                "wb_cs": wb_cs,
                "wb_dv": wb_dv.astype(np.float16),
                "wb_rs": wb_rs,
                "gk16": gk.astype(np.float16),
                "g_all": g_all.astype(np.float16),
                "wad": wad,
                "wcc": wcc,
                "smask": rowmask[:, None].copy(),
                "bsum": np.full((P, 1), bias_sum, np.float32),
            }
        )
    return in_maps


def kernel(X, weights, bias):
    if "nc" not in _CACHED:
        _CACHED["nc"] = _build_program()
    nc = _CACHED["nc"]

    trace = bool(os.environ.get("BASS_TRACE"))
    if trace:
        _install_trace_hook()

    in_maps = _host_prep(np.asarray(X), np.asarray(weights), np.asarray(bias))
    res = bass_utils.run_bass_kernel_spmd(
        nc, in_maps, core_ids=list(range(NCORES)), trace=trace
    )
    LAST_RUN_INFO.clear()
    LAST_RUN_INFO.update(
        exec_time_ns=res.exec_time_ns,
        mean_exec_time_ns=res.mean_exec_time_ns,
        trace=res.instructions_and_trace[1] if res.instructions_and_trace else None,
    )

    Yp = np.stack([np.asarray(res.results[k]["y"]) for k in range(NCORES)])
    Y = (
        Yp.astype(np.float32)
        .reshape(NCORES, G, C, RPC, MW)
        .transpose(2, 0, 3, 1, 4)
        .reshape(1, C, N, N)
    )
    return Y


# revision 59
# speedup vs baseline: 1.1046x; 1.0395x over previous
"""Equivariant layer block (order-2, 15-basis) on 8 Trainium2 NeuronCores.

Decomposition (indices: c in-channel, o out-channel, n/m spatial, N=2048):
  Y[o,n,m] = sum_c X[c,n,m] W8[c,o] + X[c,m,n] W6[c,o]
           + A[o,n] + B[o,m] + D[o,n] delta[n,m]
with (raw sums; /N factors folded into host-side weights; i = ref basis index)
  A[o,n] = dv.W5 + csum.W7/N + rsum.W12/N + dsum.W11/N + tsum.W14/N^2
  B[o,m] = dv.W9 + csum.W10/N + rsum.W13/N + sum(bias)
  D[o,n] = dv.W0 + csum.W1/N + rsum.W3/N + dsum.W2/N + tsum.W4/N^2

v4 design. Core k owns output rows I_k=[256k,256k+256). Both spatial panels
stream through a 4-chunk fp8 SBUF ring rc[(g,c), b, t, r2, m] (t=0 rows,
t=1 cols); per chunk the PE does the fused DoubleRow main matmul (identity +
transpose terms via the two k-tiles) plus identity pair-sum matmuls that
accumulate partial column sums (from the row panel) AND partial row sums
(from the column panel) in one persistent PSUM bank. The main term is
evicted table-free into a full f16 staging tile Y0 (Act/DVE alternating
plain copies), so the whole load phase is DMA-bound and the PE never
switches stationary weights mid-phase.

One f16 AllReduce of [128, 770] (pre-folded B table | csum partials | rsum
partials | masked diag column) fires as soon as the last chunk's pair-sums
land. Post-collective, a short matmul chain builds the A/B/D tables, then
pass 2 walks Y0 in 8-row blocks: two DVE 4x-mode broadcast adds (+A along
m, +B along r), a tiny Pool diag fix-up, and pipelined f16 stores on the
sync/gpsimd queues. sum(bias) is folded into B on device.
"""

import os
import numpy as np

import concourse.bacc as bacc
import concourse.tile as tile
import concourse.mybir as mybir
from concourse import bass_utils

N = 2048
C = 16
NCORES = 8
RPC = N // NCORES  # 256 rows per core
G = 8  # m-groups
MW = N // G  # 256
P = 128
CHUNK = 32  # rows per DMA chunk
NCHUNK = RPC // CHUNK  # 8
HB = CHUNK // 2  # row-pairs per chunk (16)
QR = 4  # rows per main-loop quarter-chunk
NQ = RPC // QR  # 64
BR = 8  # rows per pass-2 block
NB = RPC // BR  # 32
AXU = 8  # inner expansion of the A table (packed last dim for DVE 2x)
CCW = 2 * 256 + 256 + 2  # AllReduce payload cols: B_pre|csum|rsum|dcol|pad
f16 = mybir.dt.float16
f32 = mybir.dt.float32
f8 = mybir.dt.float8e4

LAST_RUN_INFO = {}
_CACHED = {}


def _install_trace_hook():
    """Best-effort NTFF hook injection (used only when BASS_TRACE is set)."""
    try:
        import sys, types

        if "antenv.axon_hooks" in sys.modules:
            return
        mod = types.ModuleType("antenv.axon_hooks")
        state = {}
        mod.set_axon_ntff_profile_hook = lambda h: state.update(h=h)
        mod.get_axon_ntff_profile_hook = lambda: state.get("h")
        sys.modules["antenv.axon_hooks"] = mod
        import antenv

        antenv.axon_hooks = mod
        from trn_agent_boot.trn_boot import _ntff_profile_via_ctypes

        mod.set_axon_ntff_profile_hook(
            _ntff_profile_via_ctypes("/opt/axon/libaxon_pjrt.so")
        )
    except Exception:
        pass


def _build_program():
    nc = bacc.Bacc("TRN2", target_bir_lowering=False, debug=False, num_devices=NCORES)

    # interleaved panel: rc_d[p, b, t, r2, m], row = 2b+r2, t=0 rows / t=1 cols
    rc_d = nc.dram_tensor("rc8", [P, RPC // 2, 2, 2, MW], f8, kind="ExternalInput").ap()
    wst_d = nc.dram_tensor("wst", [P, 2, P], f8, kind="ExternalInput").ap()
    id8_d = nc.dram_tensor("id8", [P, 2, P], f8, kind="ExternalInput").ap()
    ohb_d = nc.dram_tensor("ohb", [P, 2, 2, MW], f8, kind="ExternalInput").ap()
    idt_d = nc.dram_tensor("idt", [P, P], f16, kind="ExternalInput").ap()
    wbcs_d = nc.dram_tensor("wb_cs", [P, P], f32, kind="ExternalInput").ap()
    wbdv_d = nc.dram_tensor("wb_dv", [P, P], f16, kind="ExternalInput").ap()
    wbrs_d = nc.dram_tensor("wb_rs", [P, P], f32, kind="ExternalInput").ap()
    gk16_d = nc.dram_tensor("gk16", [P, C], f16, kind="ExternalInput").ap()
    gall_d = nc.dram_tensor("g_all", [P, C], f16, kind="ExternalInput").ap()
    wad_d = nc.dram_tensor("wad", [2, 96, P], f32, kind="ExternalInput").ap()
    wcc_d = nc.dram_tensor("wcc", [2, 48, P], f32, kind="ExternalInput").ap()
    smask_d = nc.dram_tensor("smask", [P, 1], f32, kind="ExternalInput").ap()
    bsum_d = nc.dram_tensor("bsum", [P, 1], f32, kind="ExternalInput").ap()

    y_d = nc.dram_tensor("y", [P, RPC, MW], f16, kind="ExternalOutput").ap()

    add = mybir.AluOpType.add
    COPY = mybir.ActivationFunctionType.Copy
    IDENT = mybir.ActivationFunctionType.Identity

    with tile.TileContext(nc) as tc:
        with (
            tc.tile_pool(name="small", bufs=1) as small,
            tc.tile_pool(name="rcring", bufs=3) as rcring,
            tc.tile_pool(name="y0p", bufs=1) as y0p,
            tc.tile_pool(name="pscr", bufs=1, space="PSUM") as pscr,
            tc.tile_pool(name="psstat", bufs=1, space="PSUM") as psstat,
            tc.tile_pool(name="psmain", bufs=3, space="PSUM") as psmain,
            tc.tile_pool(name="dram", bufs=1, space="DRAM") as dram,
        ):
            # ---- kick the first input chunks before the small weights ----
            rcts = []
            for i in range(NCHUNK):
                rct = rcring.tile([P, HB, 2, 2, MW], f8, tag="rc", name=f"rc{i}")
                rcts.append(rct)
            for i in range(2):
                nc.sync.dma_start(
                    rcts[i][:], rc_d[:, i * HB : (i + 1) * HB, :, :, :]
                )

            # ---- constant / weight loads ----
            wst = small.tile([P, 2, P], f8)
            id8 = small.tile([P, 2, P], f8)
            ohb = small.tile([P, 2, 2, MW], f8)
            idt = small.tile([P, P], f16)
            wb_cs = small.tile([P, P], f32)
            wb_dv = small.tile([P, P], f16)
            wb_rs = small.tile([P, P], f32)
            gk16 = small.tile([P, C], f16)
            g_all = small.tile([P, C], f16)
            smask = small.tile([P, 1], f32)
            bsum = small.tile([P, 1], f32)
            for t, d in [
                (wst, wst_d),
                (id8, id8_d),
                (ohb, ohb_d),
                (idt, idt_d),
                (wb_cs, wbcs_d),
                (wb_dv, wbdv_d),
                (wb_rs, wbrs_d),
                (gk16, gk16_d),
                (g_all, gall_d),
                (smask, smask_d),
                (bsum, bsum_d),
            ]:
                nc.sync.dma_start(t[:], d[:])
            wa3 = small.tile([96, P], f32)
            wd3 = small.tile([96, P], f32)
            wca2 = small.tile([48, P], f32)
            wcd2 = small.tile([48, P], f32)
            nc.sync.dma_start(wa3[:], wad_d[0])
            nc.sync.dma_start(wd3[:], wad_d[1])
            nc.sync.dma_start(wca2[:], wcc_d[0])
            nc.sync.dma_start(wcd2[:], wcc_d[1])

            Y0 = y0p.tile([P, RPC, MW], f16)  # staged main term (pre-tables)
            rdiag = small.tile([P, RPC], f16)  # diag per group (g=k rows valid)
            # csum/rsum pair-sum accumulators share one PSUM bank: [cs | rs]
            csrs = pscr.tile([P, 2 * MW], f32)
            ccbuf = small.tile([P, CCW], f16)
            gbuf = small.tile([P, CCW], f16)
            cc_in = dram.tile([P, CCW], f16)
            cc_out = dram.tile([P, CCW], f16)

            # ---- load phase: stream chunks, stats + main matmuls + evict ----
            for i in range(NCHUNK):
                r0 = i * CHUNK
                b0 = i * HB
                rct = rcts[i]
                if i >= 2:
                    nc.sync.dma_start(rct[:], rc_d[:, b0 : b0 + HB, :, :, :])
                # diag extract: row r=2b'+r2 at flat offset 1026*b' + 257*r2
                rcflat = rct.rearrange("p b t r m -> p (b t r m)")
                for r2i in range(2):
                    nc.scalar.activation(
                        rdiag[:, r0 + r2i : r0 + CHUNK : 2],
                        rcflat[
                            :,
                            257 * r2i + r0 : 257 * r2i + r0 + (HB - 1) * 1026 + 1 : 1026,
                        ],
                        COPY,
                    )
                # pair-sum stats on PE: one DoubleRow matmul per row-pair
                # covers BOTH panels (pair over r2; free dims [t, m]):
                # csrs[(g,c), t*MW+m] += rct[.., b, t, 0, m] + rct[.., b, t, 1, m]
                csrs2 = csrs.rearrange("p (t m) -> p t m", t=2)
                for b in range(HB):
                    gb = b0 + b
                    nc.tensor.matmul(
                        csrs2[:],
                        id8[:],
                        rct[:, b, :, :, :].rearrange("p t r m -> p r t m"),
                        start=(gb == 0),
                        stop=(gb == RPC // 2 - 1),
                        perf_mode=mybir.MatmulPerfMode.DoubleRow,
                        skip_group_check=True,
                    )
                # main term: one DoubleRow matmul per row-pair (pair over t)
                for s in range(CHUNK // QR):
                    qr0 = r0 + s * QR
                    pt = psmain.tile([P, QR, MW], f32, tag="pt")
                    for j in range(2):
                        nc.tensor.matmul(
                            pt[:, 2 * j : 2 * j + 2, :],
                            wst[:],
                            rct[:, 2 * s + j, :, :, :],
                            start=True,
                            stop=True,
                            perf_mode=mybir.MatmulPerfMode.DoubleRow,
                        )
                    with nc.allow_low_precision(reason="f16 staging"):
                        if s % 2 == 0:
                            nc.scalar.activation(
                                Y0[:, qr0 : qr0 + QR, :], pt[:], COPY
                            )
                        else:
                            nc.vector.tensor_copy(Y0[:, qr0 : qr0 + QR, :], pt[:])

            # ---- pre-collective fold: B_pre + payload assembly ----
            csr32 = small.tile([P, 2 * MW], f32)
            nc.scalar.activation(csr32[:], csrs[:], COPY)
            bps = psstat.tile([P, MW], f32, tag="apck")
            nc.tensor.matmul(bps[:], wb_cs[:], csr32[:, 0:MW], start=True, stop=False)
            nc.tensor.matmul(bps[:], wb_rs[:], csr32[:, MW:], start=False, stop=False)
            nc.tensor.matmul(bps[:], wb_dv[:], rdiag[:], start=False, stop=True)
            with nc.allow_low_precision(reason="f16 collective payload"):
                nc.scalar.activation(ccbuf[:, 0:MW], bps[:], COPY)
                nc.vector.tensor_copy(ccbuf[:, MW : 3 * MW], csr32[:])
                dcol = small.tile([P, 1], f32)
                nc.vector.tensor_reduce(
                    dcol[:], rdiag[:], axis=mybir.AxisListType.X, op=add
                )
                nc.vector.tensor_scalar_mul(
                    ccbuf[:, 3 * MW : 3 * MW + 1], dcol[:], smask[:]
                )
                nc.vector.memset(ccbuf[:, 3 * MW + 1 : CCW], 0.0)
            nc.gpsimd.dma_start(cc_in[:], ccbuf[:])
            # local dv stats fold overlaps the collective
            stats3 = small.tile([96, MW], f32)
            dvp = psstat.tile([P, MW], f32, tag="apck")
            nc.tensor.matmul(dvp[0:C, :], gk16[:], rdiag[:], start=True, stop=True)
            nc.scalar.activation(stats3[0:C, :], dvp[0:C, :], COPY)
            nc.gpsimd.collective_compute(
                "AllReduce",
                add,
                replica_groups=[list(range(NCORES))],
                ins=[cc_in.opt()],
                outs=[cc_out.opt()],
            )
            nc.gpsimd.dma_start(gbuf[:], cc_out[:])

            # ---- post-collective: A/B/D tables ----
            # stats3 rows: 0 dv | 16 csum | 32 rsum | 48 dsum-bcast | 64 tsum-b
            g_cs = gbuf[:, MW : 2 * MW]
            g_rs = gbuf[:, 2 * MW : 3 * MW]
            B16 = small.tile([P, MW], f16)
            with nc.allow_low_precision(reason="f16 B table"):
                nc.scalar.activation(B16[:], gbuf[:, 0:MW], IDENT, bias=bsum[:])
            csp = psstat.tile([P, MW], f32, tag="apck")
            nc.tensor.matmul(csp[0:C, :], gk16[:], g_cs, start=True, stop=True)
            nc.scalar.activation(stats3[32:48, :], csp[0:C, :], COPY)
            rsp = psstat.tile([P, MW], f32, tag="apck")
            nc.tensor.matmul(rsp[0:C, :], gk16[:], g_rs, start=True, stop=True)
            nc.scalar.activation(stats3[64:80, :], rsp[0:C, :], COPY)
            # stats2: dsum/tsum broadcast rows (32-aligned partition bases)
            stats2 = small.tile([48, MW], f32)
            dsp = psstat.tile([P, MW], f32, tag="apck")
            nc.tensor.matmul(
                dsp[0:C, 0:1], g_all[:], gbuf[:, 3 * MW : 3 * MW + 1],
                start=True, stop=True,
            )
            nc.vector.tensor_copy(
                stats2[0:C, :], dsp[0:C, 0:1].broadcast_to([C, MW])
            )
            cst2 = small.tile([P, 1], f16)
            with nc.allow_low_precision(reason="f16 total-sum scalar"):
                nc.vector.tensor_reduce(
                    cst2[:], g_cs, axis=mybir.AxisListType.X, op=add
                )
            tsp = psstat.tile([P, MW], f32, tag="apck")
            nc.tensor.matmul(tsp[0:C, 0:1], g_all[:], cst2[:], start=True, stop=True)
            nc.vector.tensor_copy(
                stats2[32:48, :], tsp[0:C, 0:1].broadcast_to([C, MW])
            )

            A16 = small.tile([P, RPC], f16)
            aps = psstat.tile([P, MW], f32, tag="apck")
            nc.tensor.matmul(aps[:], wa3[:], stats3[:], start=True, stop=False)
            nc.tensor.matmul(aps[:], wca2[:], stats2[:], start=False, stop=True)
            with nc.allow_low_precision(reason="f16 A table"):
                nc.scalar.activation(A16[:], aps[:], COPY)
            # A expanded AXU-wide so pass-2 keeps a packed last dim (DVE 4x)
            A8e = small.tile([P, RPC, AXU], f16)
            with nc.allow_low_precision(reason="f16 A table"):
                nc.vector.tensor_copy(
                    A8e[:],
                    A16.rearrange("p (n x) -> p n x", x=1).broadcast_to(
                        [P, RPC, AXU]
                    ),
                )
            Dm16 = small.tile([P, RPC], f16)
            dps = psstat.tile([P, MW], f32, tag="apck")
            nc.tensor.matmul(dps[:], wd3[:], stats3[:], start=True, stop=False)
            nc.tensor.matmul(dps[:], wcd2[:], stats2[:], start=False, stop=True)
            with nc.allow_low_precision(reason="f16 D table"):
                nc.scalar.activation(Dm16[:], dps[:], COPY)
            # A16b = A + sum(bias): per-row bias for the PE-path evictions
            A16b = small.tile([P, RPC], f16)
            with nc.allow_low_precision(reason="f16 A table"):
                nc.scalar.activation(A16b[:], A16[:], IDENT, bias=bsum[:])
            # bt8: fp8 transpose of the bias-free B table, x32 scaled so the
            # small B values stay in fp8e4m3 normal range (ohb carries 1/32)
            bt8 = small.tile([P, 2, P], f8)
            for mb in range(2):
                btp = psstat.tile([P, P], f16, tag="apck")
                nc.tensor.matmul(
                    btp[:],
                    gbuf[:, mb * P : (mb + 1) * P],
                    idt[:],
                    is_transpose=True,
                    start=True,
                    stop=True,
                    skip_group_check=True,
                )
                with nc.allow_low_precision(reason="fp8 B table"):
                    nc.scalar.activation(bt8[:, mb, :], btp[:], COPY, scale=32.0)

            # ---- diag fix-up: one strided add over ALL of Y0 (row r diag
            # sits at flat r*257 within [r, m']; Dm16 is zero off g=k) ----
            y0flat = Y0.rearrange("p n m -> p (n m)")
            with nc.allow_low_precision(reason="f16 output"):
                nc.gpsimd.tensor_tensor(
                    y0flat[:, 0 : (RPC - 1) * (MW + 1) + 1 : MW + 1],
                    y0flat[:, 0 : (RPC - 1) * (MW + 1) + 1 : MW + 1],
                    Dm16[:],
                    op=add,
                )

            # ---- pass 2: +A (bcast m), +B (bcast r) in place on Y0, store --
            # DVE-path blocks: two 2x-mode f16 adds. PE-path blocks: identity
            # copy-matmul + one-hot B matmul into PSUM, Act per-row bias=A
            # eviction. Split keeps the tail under the store-DMA bound.
            pe_path = {qq for qq in range(NB) if qq % 5 in (1, 3)}
            for qq in range(NB):
                r0 = qq * BR
                y0b = Y0[:, r0 : r0 + BR, :]
                if qq in pe_path:
                    # PE path: 2 q-subblocks of 4 rows
                    pts = []
                    for j in range(2):
                        pt2 = psmain.tile([P, QR, MW], f32, tag="pt")
                        for h in range(2):
                            nc.tensor.matmul(
                                pt2[:, 2 * h : 2 * h + 2, :],
                                idt[:],
                                y0b[:, 4 * j + 2 * h : 4 * j + 2 * h + 2, :],
                                start=True,
                                stop=False,
                                skip_group_check=True,
                            )
                        pts.append(pt2)
                    for j in range(2):
                        for h in range(2):
                            nc.tensor.matmul(
                                pts[j][:, 2 * h : 2 * h + 2, :],
                                bt8[:],
                                ohb[:],
                                start=False,
                                stop=True,
                                perf_mode=mybir.MatmulPerfMode.DoubleRow,
                                skip_group_check=True,
                            )
                    with nc.allow_low_precision(reason="f16 output"):
                        for r in range(BR):
                            nc.scalar.activation(
                                y0b[:, r, :],
                                pts[r // 4][:, r % 4, :],
                                IDENT,
                                bias=A16b[:, r0 + r : r0 + r + 1],
                            )
                else:
                    y0b4 = y0b.rearrange("p n (u x) -> p n u x", x=AXU)
                    with nc.allow_low_precision(reason="f16 output"):
                        nc.vector.tensor_tensor(
                            y0b4[:],
                            y0b4[:],
                            A8e[:, r0 : r0 + BR, :]
                            .rearrange("p n (u x) -> p n u x", u=1)
                            .broadcast_to([P, BR, MW // AXU, AXU]),
                            op=add,
                        )
                        nc.vector.tensor_tensor(
                            y0b[:],
                            y0b[:],
                            B16.rearrange("p (n m) -> p n m", n=1).broadcast_to(
                                [P, BR, MW]
                            ),
                            op=add,
                        )
                eng = nc.sync if qq % 2 == 0 else nc.gpsimd
                eng.dma_start(y_d[:, r0 : r0 + BR, :], y0b[:])

    nc.compile()
    return nc


def _host_prep(X, weights, bias):
    """Pack panels + fold weights into per-core input maps."""
    import ml_dtypes

    W = weights.astype(np.float32)
    iN = np.float32(1.0 / N)
    iN2 = np.float32(1.0 / (N * N))
    bias_sum = np.float32(bias.astype(np.float64).sum())

    Xr = np.ascontiguousarray(X[0])  # [C, N, N] fp32
    Rp = (
        Xr.reshape(C, NCORES, RPC, G, MW)
        .transpose(1, 3, 0, 2, 4)
        .reshape(NCORES, P, RPC, MW)
        .astype(ml_dtypes.float8_e4m3)
    )
    XT = np.ascontiguousarray(Xr.transpose(0, 2, 1))
    Cp = (
        XT.reshape(C, NCORES, RPC, G, MW)
        .transpose(1, 3, 0, 2, 4)
        .reshape(NCORES, P, RPC, MW)
        .astype(ml_dtypes.float8_e4m3)
    )
    # interleave: rc8[k, p, b, t, r2, m], row = 2b + r2; t=0 rows, t=1 cols
    rc8 = np.ascontiguousarray(
        np.stack(
            [
                Rp.reshape(NCORES, P, RPC // 2, 2, MW),
                Cp.reshape(NCORES, P, RPC // 2, 2, MW),
            ],
            axis=3,
        )
    )

    def blockdiag(w, dtype):
        out = np.zeros((P, P), dtype=dtype)
        for g in range(G):
            out[g * C : (g + 1) * C, g * C : (g + 1) * C] = w
        return out

    wst = np.stack(
        [
            blockdiag(W[8], np.float32),
            blockdiag(W[6], np.float32),
        ],
        axis=1,
    ).astype(ml_dtypes.float8_e4m3)  # [128, 2, 128]

    id8 = np.stack([np.eye(P, dtype=np.float32)] * 2, axis=1).astype(
        ml_dtypes.float8_e4m3
    )  # [128, 2, 128] stacked identities for the pair-sum matmuls
    idt = np.eye(P, dtype=np.float16)
    # one-hot B pattern (x 1/32; bt8 is x32): ohb[k, t, r, m] = (m == 128t+k)/32
    ohb = np.zeros((P, 2, 2, N // G), np.float32)
    for t in range(2):
        for k in range(P):
            ohb[k, t, :, t * P + k] = 1.0 / 32.0
    ohb = ohb.astype(ml_dtypes.float8_e4m3)
    wb_cs = blockdiag(W[10] * iN, np.float32)
    wb_rs = blockdiag(W[13] * iN, np.float32)
    g_all = np.tile(np.eye(C, dtype=np.float32), (G, 1))  # [128, 16]

    def rep(w):  # [16,16] -> [16, 128]
        return np.tile(w.astype(np.float32), (1, G))

    in_maps = []
    for k in range(NCORES):
        rowmask = np.repeat((np.arange(G) == k).astype(np.float32), C)  # [128]
        gk = g_all * rowmask[:, None]
        wb_dv = blockdiag(W[9], np.float32) * rowmask[:, None]
        # stats3 rows: 0 dv | 32 csum | 64 rsum; stats2: 0 dsum-b | 32 tsum-b
        wad = np.zeros((2, 96, P), np.float32)
        wad[0, 0:C] = rep(W[5])
        wad[0, 32:48] = rep(W[7] * iN)
        wad[0, 64:80] = rep(W[12] * iN)
        wad[1, 0:C] = rep(W[0]) * rowmask[None, :]
        wad[1, 32:48] = rep(W[1] * iN) * rowmask[None, :]
        wad[1, 64:80] = rep(W[3] * iN) * rowmask[None, :]
        wcc = np.zeros((2, 48, P), np.float32)
        wcc[0, 0:C] = rep(W[11] * iN)
        wcc[0, 32:48] = rep(W[14] * iN2)
        wcc[1, 0:C] = rep(W[2] * iN) * rowmask[None, :]
        wcc[1, 32:48] = rep(W[4] * iN2) * rowmask[None, :]
        in_maps.append(
            {
                "rc8": rc8[k],
                "wst": wst,
                "id8": id8,
                "idt": idt,
                "ohb": ohb,
                "wb_cs": wb_cs,
                "wb_dv": wb_dv.astype(np.float16),
                "wb_rs": wb_rs,
                "gk16": gk.astype(np.float16),
                "g_all": g_all.astype(np.float16),
                "wad": wad,
                "wcc": wcc,
                "smask": rowmask[:, None].copy(),
                "bsum": np.full((P, 1), bias_sum, np.float32),
            }
        )
    return in_maps


def kernel(X, weights, bias):
    if "nc" not in _CACHED:
        _CACHED["nc"] = _build_program()
    nc = _CACHED["nc"]

    trace = bool(os.environ.get("BASS_TRACE"))
    if trace:
        _install_trace_hook()

    in_maps = _host_prep(np.asarray(X), np.asarray(weights), np.asarray(bias))
    res = bass_utils.run_bass_kernel_spmd(
        nc, in_maps, core_ids=list(range(NCORES)), trace=trace
    )
    LAST_RUN_INFO.clear()
    LAST_RUN_INFO.update(
        exec_time_ns=res.exec_time_ns,
        mean_exec_time_ns=res.mean_exec_time_ns,
        trace=res.instructions_and_trace[1] if res.instructions_and_trace else None,
    )

    Yp = np.stack([np.asarray(res.results[k]["y"]) for k in range(NCORES)])
    Y = (
        Yp.astype(np.float32)
        .reshape(NCORES, G, C, RPC, MW)
        .transpose(2, 0, 3, 1, 4)
        .reshape(1, C, N, N)
    )
    return Y
